# revision 6
# baseline (speedup 1.0000x reference)
"""BitConv2d (ternary-weight 3x3 conv, power-of-two rescale) on 8 TRN2 NeuronCores.

Strategy:
  - Data-parallel over batch: 32 images -> 4 per core (2 image pairs).
  - Activation quantization x_int = clip(round(clip(x,-1,1)/2^-6), -127, 127)
    is computed exactly with f32 engine ops:
      i16 = RNE(64*x + 128) on GPSIMD (hw f32->i16 cast rounds to nearest even),
      bf16 = clip(i16, 64, 192) on DVE -> v = x_int + 128 (exact ints in bf16).
    The +128 offset keeps values positive; padded border cells are memset to
    128 so the offset contributes exactly 128*sum(w) per output channel,
    which is folded into the bias on the host.
  - Conv as 9 accumulating matmuls per output tile (K=Cin=64, M=Cout=64),
    packed 4-per-array with tile_position quadrants:
      rows 0-63   = image A channels, rows 64-127 = image B channels
      cols 0-63   = output row-block r, cols 64-127 = row-block r+1.
    PSUM pairing is per image: ps_A[0:64] = (A, blk r), ps_A[64:128] =
    (A, blk r+1) so the epilogue + store run on full 128-partition tiles.
  - Epilogue y = psum * 2^(act_exp+s_exp[c]) + bias'[c] -> fp16 (error
    ~2^-11 relative, well under tolerance); image A on DVE, image B on ACT.
  - DMA split across both HWDGE rings: image-A loads + A stores on the sync
    engine ring (qSyncDynamicHW), image-B loads + B stores on the scalar
    engine ring (qActDynamicHW) so the two rings drain concurrently.
Output returned as float32 (host upcast of the fp16 device output).
"""

import numpy as np
import ml_dtypes
from contextlib import ExitStack

_NC_CACHE = {}

N_CORES = 8
H = W = 112
HP = H + 2  # padded
CIN = COUT = 64
P = 128
IMGS_PER_CORE = 4
ROWS_PER_CHUNK = 8            # quantization chunk (input rows)
ROWS_PER_TILE = 4             # output rows per matmul tile (N = 4*112 = 448)
NFREE = ROWS_PER_TILE * W     # 448


def _patch_tile_drain(tile_mod):
    """This walrus build rejects a Drain carrying many sync waits; split the
    final Tile drain into single-wait sync nops."""
    from concourse.vector_clock import ScopedClock, VectorClock

    if getattr(tile_mod.TileContext, "_drain_patched", False):
        return

    def _drain_and_barrier_split(self, tick_clock, wait_clock):
        vclock = tick_clock.global_clock
        n = len(vclock)
        for proc in range(n):
            t = vclock[proc]
            if t <= 0:
                continue
            vec = [0] * n
            vec[proc] = t
            nop = self.nc.sync.nop()
            wait_clock.add_sem_waits(nop.ins, ScopedClock({None: VectorClock(vec)}))
        self.nc.sync.drain()
        assert self.sems is not None
        popped = self.nc._tile_sem_poison_stack.pop()
        assert popped is self._sem_poison
        self.nc.all_engine_barrier()
        self.nc.clear_and_free_semaphores(list(self.sems.allocated().values()))
        self.nc.all_engine_barrier()

    tile_mod.TileContext._drain_and_barrier = _drain_and_barrier_split
    tile_mod.TileContext._drain_patched = True


def _split_multi_syncs(nc):
    """This walrus build accepts at most ONE sync wait (and one update) per
    instruction.  Hoist extra waits onto preceding nops and extra updates onto
    following nops (same engine, so ordering semantics are preserved)."""
    import concourse.mybir as mybir

    fn = nc.m.functions[0]
    ctr = 0
    for bb in fn.blocks:
        new_insts = []
        for inst in bb.instructions:
            si = inst.sync_info
            pre, post = [], []
            if si is not None and si.on_wait and len(si.on_wait) > 1:
                for w in list(si.on_wait[:-1]):
                    ctr += 1
                    pre.append(
                        mybir.InstNoOp(
                            name=f"wsplit_nop_{ctr}",
                            engine=inst.engine,
                            sync_info=mybir.SyncInfo(on_wait=[w], on_update=[]),
                        )
                    )
                si.on_wait = [si.on_wait[-1]]
            if (
                si is not None
                and si.on_update
                and len(si.on_update) > 1
                and not isinstance(inst, (mybir.InstDMACopy, mybir.InstDMA))
            ):
                for u in list(si.on_update[1:]):
                    ctr += 1
                    post.append(
                        mybir.InstNoOp(
                            name=f"usplit_nop_{ctr}",
                            engine=inst.engine,
                            sync_info=mybir.SyncInfo(on_wait=[], on_update=[u]),
                        )
                    )
                si.on_update = [si.on_update[0]]
            new_insts.extend(pre)
            new_insts.append(inst)
            new_insts.extend(post)
        if len(new_insts) != len(bb.instructions):
            bb.instructions[:] = new_insts
    for bb in fn.blocks:
        for inst in bb.instructions:
            if inst.name.startswith(("wsplit_nop_", "usplit_nop_")):
                if inst.name not in nc.inst_map:
                    nc.register_instruction(inst)
    return ctr


def build_nc(repeat: int = 1):
    import concourse.bass as bass
    import concourse.mybir as mybir
    import concourse.tile as tile

    _patch_tile_drain(tile)

    f32 = mybir.dt.float32
    f16 = mybir.dt.float16
    bf16 = mybir.dt.bfloat16
    i16 = mybir.dt.int16
    Alu = mybir.AluOpType
    Act = mybir.ActivationFunctionType

    nc = bass.Bass(trn_type="TRN2")
    x4 = nc.dram_tensor("x4", (IMGS_PER_CORE, CIN, H, W), f32, kind="ExternalInput")
    wsb = nc.dram_tensor("wsb", (P, 9 * COUT), bf16, kind="ExternalInput")
    sb = nc.dram_tensor("sb", (P, 2), f32, kind="ExternalInput")
    y4 = nc.dram_tensor("y4", (IMGS_PER_CORE, COUT, H, W), f16, kind="ExternalOutput")

    n_chunks = H // ROWS_PER_CHUNK          # 14 quant chunks of 8 rows
    n_iters = H // (2 * ROWS_PER_TILE)      # 14 conv iterations (8 rows each)
    HR = 58                                 # rows per xq half-tile (padded)

    with tile.TileContext(nc) as tc, ExitStack() as ctx:
        const_pool = ctx.enter_context(tc.tile_pool(name="const", bufs=1))
        xq_pool = ctx.enter_context(tc.tile_pool(name="xq", bufs=2))
        stg_pool = ctx.enter_context(tc.tile_pool(name="stg", bufs=4))
        rnd_pool = ctx.enter_context(tc.tile_pool(name="rnd", bufs=4))
        out_pool = ctx.enter_context(tc.tile_pool(name="out", bufs=8))
        psum_pool = ctx.enter_context(
            tc.tile_pool(name="psum", bufs=8, space=bass.MemorySpace.PSUM)
        )

        w_t = const_pool.tile([P, 9 * COUT], bf16)
        nc.sync.dma_start(w_t[:], wsb[:])
        sb_t = const_pool.tile([P, 2], f32)
        nc.sync.dma_start(sb_t[:], sb[:])

        # sequence of image pairs (repeat only multiplies work for timing)
        seq = []
        for rep in range(repeat):
            for pr in range(IMGS_PER_CORE // 2):
                seq.append((2 * pr, 2 * pr + 1))

        def alloc_xq():
            # split padded image vertically: top = padded rows 0..57,
            # bottom = padded rows 56..113 (2-row halo overlap).  Finer
            # dependency granularity lets conv start after half the quant.
            xqt = xq_pool.tile([P, HR, HP], bf16)
            xqb = xq_pool.tile([P, HR, HP], bf16)
            nc.vector.memset(xqt[:, 0, :], 128.0)
            nc.vector.memset(xqt[:, :, 0], 128.0)
            nc.vector.memset(xqt[:, :, HP - 1], 128.0)
            nc.vector.memset(xqb[:, HR - 1, :], 128.0)
            nc.vector.memset(xqb[:, :, 0], 128.0)
            nc.vector.memset(xqb[:, :, HP - 1], 128.0)
            return xqt, xqb

        def emit_quant(pair, tiles, ch):
            img_a, img_b = pair
            xqt, xqb = tiles
            r0 = ch * ROWS_PER_CHUNK
            r1 = r0 + ROWS_PER_CHUNK - 1
            stg = stg_pool.tile([P, ROWS_PER_CHUNK, W], f32)
            nc.sync.dma_start(stg[0:64], x4[img_a, :, r0:r1 + 1, :])
            nc.sync.dma_start(stg[64:128], x4[img_b, :, r0:r1 + 1, :])
            rnd = rnd_pool.tile([P, ROWS_PER_CHUNK, W], i16)
            # i16 = RNE(64*x + 128): hw f32->i16 cast rounds to nearest even
            # (gpsimd: fast for contiguous ops; keeps DVE/ACT free for epilogue)
            nc.gpsimd.tensor_scalar(out=rnd[:], in0=stg[:], scalar1=64.0,
                                    scalar2=128.0, op0=Alu.mult, op1=Alu.add)
            # bf16 = clip(i16, 64, 192) == x_int + 128, written into padded halves
            segs = []
            ta, tb = max(r0, 0), min(r1, 56)       # top covers img rows 0..56
            if ta <= tb:
                segs.append((xqt, ta + 1, ta - r0, tb - ta + 1))
            ba, bb = max(r0, 55), r1               # bottom covers img rows 55..112
            if ba <= bb:
                segs.append((xqb, ba - 55, ba - r0, bb - ba + 1))
            for t, dst0, src0, nrows in segs:
                nc.vector.tensor_scalar(
                    out=t[:, dst0:dst0 + nrows, 1:1 + W],
                    in0=rnd[:, src0:src0 + nrows, :],
                    scalar1=64, scalar2=192, op0=Alu.max, op1=Alu.min,
                )

        def emit_conv_iter(pair, tiles, it):
            img_a, img_b = pair
            r0 = it * 2 * ROWS_PER_TILE
            if it < 7:
                xq, base = tiles[0], r0          # top-local row == padded row
            else:
                xq, base = tiles[1], r0 - 56     # bottom-local = padded - 56
            # ps[0] = image A (blk r in parts 0-63, blk r+1 in parts 64-127),
            # ps[1] = image B likewise.
            ps = []
            for _q in range(2):
                pq = psum_pool.tile([P, NFREE], f32)
                ps.append(pq)
            for tap in range(9):
                dh, dw = divmod(tap, 3)
                st, sp = tap == 0, tap == 8
                for n_img in range(2):           # array row half (image)
                    r = 64 * n_img
                    for blk in range(2):         # array col half (row block)
                        c = 64 * blk
                        hs = base + ROWS_PER_TILE * blk + dh
                        nc.tensor.matmul(
                            ps[n_img][c:c + 64, :],
                            w_t[r:r + 64, tap * 64:(tap + 1) * 64],
                            xq[r:r + 64, hs:hs + ROWS_PER_TILE, dw:dw + W],
                            start=st, stop=sp,
                        )
            for n_img, img in enumerate((img_a, img_b)):
                o = out_pool.tile([P, ROWS_PER_TILE, W], f16)
                if n_img == 0:
                    nc.vector.tensor_scalar(
                        out=o[:], in0=ps[0],
                        scalar1=sb_t[:, 0:1], scalar2=sb_t[:, 1:2],
                        op0=Alu.mult, op1=Alu.add,
                    )
                else:
                    nc.scalar.activation(
                        o[:], ps[1], Act.Identity,
                        scale=sb_t[:, 0:1], bias=sb_t[:, 1:2],
                    )
                # one 8-row store per image: partitions (blk, ch) -> dram
                # (blk, ch, 4 rows, W) via a 4-D dram access pattern
                dst = y4[img, :, r0:r0 + 2 * ROWS_PER_TILE, :].rearrange(
                    "c (b r) w -> b c r w", b=2
                )
                nc.scalar.dma_start(dst, o[:])

        # software pipeline: conv(pair k) interleaves with quant(pair k+1)
        tiles_k = alloc_xq()
        for ch in range(n_chunks):
            emit_quant(seq[0], tiles_k, ch)
        for k, pair in enumerate(seq):
            tiles_next = alloc_xq() if k + 1 < len(seq) else None
            for it in range(n_iters):
                if tiles_next is not None:
                    emit_quant(seq[k + 1], tiles_next, it)
                emit_conv_iter(pair, tiles_k, it)
            tiles_k = tiles_next

    _split_multi_syncs(nc)
    nc.finalize()
    return nc


def _host_prep(w_q, s_exp, bias, act_exp):
    """Weights in lhsT layout (dup on both partition halves) + scale/bias fold."""
    w_half = np.transpose(w_q, (1, 2, 3, 0)).reshape(CIN, 9 * COUT)  # [ci, tap*64+co]
    wsb = np.concatenate([w_half, w_half], axis=0).astype(ml_dtypes.bfloat16)

    s_exp = np.asarray(s_exp).reshape(-1).astype(np.float64)
    scale = np.exp2(float(act_exp) + s_exp)                       # [64]
    wsum = w_q.astype(np.float64).sum(axis=(1, 2, 3))             # [64]
    bias_c = np.asarray(bias).astype(np.float64) - 128.0 * wsum * scale
    col_scale = np.tile(scale, 2).astype(np.float32)
    col_bias = np.tile(bias_c, 2).astype(np.float32)
    sb = np.stack([col_scale, col_bias], axis=1)                  # [128, 2] f32
    return wsb, sb


def kernel(x, w_q, s_exp, bias, act_exp):
    from concourse.bass_utils import run_bass_kernel_spmd

    x = np.ascontiguousarray(np.asarray(x, dtype=np.float32))
    wsb, sb = _host_prep(np.asarray(w_q), s_exp, bias, int(act_exp))

    if "nc" not in _NC_CACHE:
        _NC_CACHE["nc"] = build_nc()
    nc = _NC_CACHE["nc"]

    in_maps = [
        {"x4": x[4 * c:4 * c + 4], "wsb": wsb, "sb": sb}
        for c in range(N_CORES)
    ]
    res = run_bass_kernel_spmd(nc, in_maps, core_ids=list(range(N_CORES)))
    out = np.concatenate([res.results[c]["y4"] for c in range(N_CORES)], axis=0)
    return out.astype(np.float32)
